# revision 6
# baseline (speedup 1.0000x reference)
"""CombinePatches (3D col2im fold + overlap-count normalize) on 8 TRN2 NeuronCores.

Decomposition (validated numerically against the reference):
  out[b, 2q+kd, 2s+kh, 2u+kw, c] (+)= patches[b, q, s, u, kd, kh, kw, c], then
  out /= cnt, cnt = cd(d)*ch(h)*cw(w) separable overlap counts.

Sharding: 8 cores = B(2) x D-chunks(4). Each core computes 16 output d-rows from
9 od-slices of patches (1 halo slice, zero-padded at global edges by the host).

v2 design (HBM-bound problem; measured baseline streamed fp32 at the 358 GB/s
per-core roofline, so the wins are fewer bytes + shorter tail):
  - patch stream in bf16 (halves input DMA; end-to-end rel err ~3e-3, validated
    against the reference in numpy with exact-layout simulation).
  - SBUF slice layout [p = kh_lo*63 + s, (kh_hi, kd, kw_pair, x=u+1, kw_lo, c)]:
    the w-fold becomes a fully contiguous 512-elem DVE add (no strided gather),
    and the h-fold contracts K=(kh_lo, s)=126 packed partitions, so each output
    row needs only 4 matmuls of N=512 (PE max) instead of 16 of N=256.
  - 0.25*rh(h) (interior rd * interior rw * exact rh) baked into the weights;
    host rescales the 4 global d-edge rows and w-edge columns by 2 after gather.
  - evict/store alternate between ACT and Pool engines so the drain never
    serializes on one engine; slice loads stream on the sync ring from 9
    statically allocated SBUF tiles (no pool-slot release in the DMA path).
"""
import sys

for _p in ("/opt/trn_rl_repo", "/opt/trn_rl_repo/pypackages"):
    if _p not in sys.path:
        sys.path.insert(0, _p)

from contextlib import ExitStack

import numpy as np
import ml_dtypes

import concourse.bass as bass
import concourse.tile as tile
from concourse import bacc, mybir
from concourse import bass_utils

B, D, H, W, C = 2, 64, 128, 128, 4
od, oh, ow = 31, 63, 63
NS = 9              # od-slices per core (incl 1 halo)
RPC = 16            # output d-rows per core
X = 65              # x slots (x = u+1; pads at x=0 and x=64)
WFULL = 2 * 4 * 2 * X * 8   # (pr, kd, vg, x, vh, c) free width, full slice
WHALF = WFULL // 2          # half slices carry only 2 kd values
BF = mybir.dt.bfloat16

_cache = {}


def _slice_width(k):
    return WHALF if k in (0, NS - 1) else WFULL


_OFFS = np.concatenate([[0], np.cumsum([126 * _slice_width(k) for k in range(NS)])])
PP_TOTAL = int(_OFFS[-1])


def _build():
    nc = bacc.Bacc(
        "TRN2",
        target_bir_lowering=False,
        debug=False,
        enable_asserts=False,
        num_devices=8,
    )
    pp_d = nc.dram_tensor("pp", [PP_TOTAL], BF, kind="ExternalInput").ap()
    wm_d = nc.dram_tensor("wm", [126, 256], BF, kind="ExternalInput").ap()
    out_d = nc.dram_tensor("out", [RPC, H, W * C], BF, kind="ExternalOutput").ap()

    with ExitStack() as ctx:
        tc = ctx.enter_context(tile.TileContext(nc))
        const_pool = ctx.enter_context(tc.tile_pool(name="const", bufs=1))
        slice_pool = ctx.enter_context(tc.tile_pool(name="slice", bufs=NS))
        t_pool = ctx.enter_context(tc.tile_pool(name="tt", bufs=8))
        ev_pool = ctx.enter_context(tc.tile_pool(name="ev", bufs=4))
        psum_pool = ctx.enter_context(tc.tile_pool(name="ps", bufs=4, space="PSUM"))

        # weights on the scalar ring so the sync ring is purely slice loads
        wm_sb = const_pool.tile([126, 256], BF)
        nc.scalar.dma_start(wm_sb[:], wm_d[:])

        # statically allocated slice tiles: all 9 loads issue immediately and
        # stream back-to-back on the sync ring at full HBM bandwidth
        def wfold(tk, nkd, ki, rr):
            """One 1024-wide add producing T[p, (pr, a, t, c)]:
            T = P[u=a, kw=t] + P[u=a-1, kw=t+2], contiguous 512-runs per pr."""
            v3 = tk[:].rearrange("p (pr rest) -> p pr rest", pr=2)
            T = t_pool.tile([126, 1024], BF, tag="T")
            T3 = T[:].rearrange("p (pr f) -> p pr f", pr=2)
            eng = nc.vector if rr == 0 else nc.gpsimd
            eng.tensor_add(
                T3,
                v3[:, :, ki * 1040 + 8 : ki * 1040 + 520],
                v3[:, :, ki * 1040 + 520 : ki * 1040 + 1032],
            )
            return T

        def mm_half(ps, T, start, stop):
            # h-fold: K=(kh_lo, s)=126 packed; N=512 (PE max) per matmul
            for pr in range(2):
                nc.tensor.matmul(
                    ps[:],
                    wm_sb[:, pr * 128 : (pr + 1) * 128],
                    T[:, pr * 512 : (pr + 1) * 512],
                    start=start and pr == 0,
                    stop=stop and pr == 1,
                )

        # software pipeline over slices: when slice k lands, finish rows
        # (k-1 pair) with their A-half (adds + matmuls + evict + store), then
        # immediately start the B-half of the next rows so only the A-half of
        # the final two rows sits on the tail critical path.
        tiles = []
        pend = {}  # rr -> (ps, d_loc)
        for k in range(NS):
            w = _slice_width(k)
            t = slice_pool.tile([126, w], BF, tag="slice")
            src = pp_d[int(_OFFS[k]) : int(_OFFS[k]) + 126 * w].rearrange(
                "(p f) -> p f", f=w
            )
            nc.sync.dma_start(t[:], src)
            tiles.append(t)

            if k >= 1:
                nkd_A = 2 if k == NS - 1 else 4
                for rr in range(2):
                    ps, d_loc = pend[rr]
                    TA = wfold(tiles[k], nkd_A, rr, rr)
                    mm_half(ps, TA, start=False, stop=True)
                    ev = ev_pool.tile([128, 512], BF, tag="ev")
                    if rr == 0:
                        nc.scalar.copy(ev[:], ps[:])
                    else:
                        nc.vector.tensor_copy(ev[:], ps[:])
                    nc.scalar.dma_start(out_d[d_loc], ev[:])
            if k <= NS - 2:
                nkd_B = 2 if k == 0 else 4
                for rr in range(2):
                    ps = psum_pool.tile([128, 512], mybir.dt.float32, tag="ps")
                    kiB = rr if k == 0 else rr + 2
                    TB = wfold(tiles[k], nkd_B, kiB, rr)
                    mm_half(ps, TB, start=True, stop=False)
                    pend[rr] = (ps, 2 * k + rr)
    nc.compile()
    return nc


def _host_wm():
    rh = np.where(
        (np.arange(H) < 2) | (np.arange(H) >= H - 2), 1.0, 0.5
    ).astype(np.float32)
    wm = np.zeros((126, 256), np.float32)
    jj = np.arange(2)[:, None]
    s = np.arange(63)[None, :]
    for pr in range(2):
        h = (2 * s + 2 * pr + jj).ravel()
        wm[np.arange(126), pr * 128 + h] = 0.25 * rh[h]
    return wm.astype(ml_dtypes.bfloat16)


def _shard_inputs(patches):
    """Per-core flat bf16 patch blocks, 9 slices each in layout
    [p = kh_lo*63 + s, (kh_hi, kd, kw_pair, x=u+1, kw_lo, c)]."""
    P5 = np.asarray(patches, np.float32).reshape(B, od, oh, ow, 4, 4, 4, 4)
    P5 = P5.astype(ml_dtypes.bfloat16)
    pps = []
    for core in range(8):
        b, kc = core // 4, core % 4
        parts = []
        for k in range(NS):
            q = 8 * kc - 1 + k
            kdl = slice(2, 4) if k == 0 else slice(0, 2) if k == NS - 1 else slice(0, 4)
            nkd = 2 if k in (0, NS - 1) else 4
            arr = np.zeros((2, 63, 2, nkd, 2, X, 2, 4), ml_dtypes.bfloat16)
            if 0 <= q < od:
                src = P5[b, q, :, :, kdl]                      # s,u,kd',kh,kw,c
                s6 = src.reshape(63, 63, nkd, 2, 2, 2, 2, 4)   # s,u,kd,pr,jj,vg,vh,c
                arr[:, :, :, :, :, 1:64] = s6.transpose(4, 0, 3, 2, 5, 1, 6, 7)
            parts.append(arr.reshape(-1))
        pps.append(np.concatenate(parts))
    return pps


def _run(patches, trace=False):
    if "nc" not in _cache:
        _cache["nc"] = _build()
        _cache["wm"] = _host_wm()
    nc = _cache["nc"]
    wm = _cache["wm"]
    pps = _shard_inputs(patches)
    in_maps = [{"pp": pps[core], "wm": wm} for core in range(8)]
    res = bass_utils.run_bass_kernel_spmd(
        nc, in_maps, core_ids=list(range(8)), trace=trace
    )
    out = np.zeros((B, D, H, W, C), np.float32)
    for core in range(8):
        b, kc = core // 4, core % 4
        r = np.asarray(res.results[core]["out"]).astype(np.float32)
        out[b, RPC * kc : RPC * (kc + 1)] = r.reshape(RPC, H, W, C)
    out[:, [0, 1, D - 2, D - 1]] *= 2.0
    out[:, :, :, [0, 1, W - 2, W - 1], :] *= 2.0
    return out, res


def kernel(patches, inputs):
    out, _ = _run(patches)
    return out


# revision 7
# speedup vs baseline: 1.0359x; 1.0359x over previous
"""CombinePatches (3D col2im fold + overlap-count normalize) on 8 TRN2 NeuronCores.

Decomposition (validated numerically against the reference):
  out[b, 2q+kd, 2s+kh, 2u+kw, c] (+)= patches[b, q, s, u, kd, kh, kw, c], then
  out /= cnt, cnt = cd(d)*ch(h)*cw(w) separable overlap counts.

Sharding: 8 cores = B(2) x D-chunks(4). Each core computes 16 output d-rows from
9 od-slices of patches (1 halo slice, zero-padded at global edges by the host).

v2 design (HBM-bound problem; measured baseline streamed fp32 at the 358 GB/s
per-core roofline, so the wins are fewer bytes + shorter tail):
  - patch stream in bf16 (halves input DMA; end-to-end rel err ~3e-3, validated
    against the reference in numpy with exact-layout simulation).
  - SBUF slice layout [p = kh_lo*63 + s, (kh_hi, kd, kw_pair, x=u+1, kw_lo, c)]:
    the w-fold becomes a fully contiguous 512-elem DVE add (no strided gather),
    and the h-fold contracts K=(kh_lo, s)=126 packed partitions, so each output
    row needs only 4 matmuls of N=512 (PE max) instead of 16 of N=256.
  - 0.25*rh(h) (interior rd * interior rw * exact rh) baked into the weights;
    host rescales the 4 global d-edge rows and w-edge columns by 2 after gather.
  - evict/store alternate between ACT and Pool engines so the drain never
    serializes on one engine; slice loads stream on the sync ring from 9
    statically allocated SBUF tiles (no pool-slot release in the DMA path).
"""
import sys

for _p in ("/opt/trn_rl_repo", "/opt/trn_rl_repo/pypackages"):
    if _p not in sys.path:
        sys.path.insert(0, _p)

from contextlib import ExitStack

import numpy as np
import ml_dtypes

import concourse.bass as bass
import concourse.tile as tile
from concourse import bacc, mybir
from concourse import bass_utils

B, D, H, W, C = 2, 64, 128, 128, 4
od, oh, ow = 31, 63, 63
NS = 9              # od-slices per core (incl 1 halo)
RPC = 16            # output d-rows per core
X = 65              # x slots (x = u+1; pads at x=0 and x=64)
WFULL = 2 * 4 * 2 * X * 8   # (pr, kd, vg, x, vh, c) free width, full slice
WHALF = WFULL // 2          # half slices carry only 2 kd values
BF = mybir.dt.bfloat16

_cache = {}


def _slice_width(k):
    return WHALF if k in (0, NS - 1) else WFULL


_OFFS = np.concatenate([[0], np.cumsum([126 * _slice_width(k) for k in range(NS)])])
PP_TOTAL = int(_OFFS[-1])


def _build():
    nc = bacc.Bacc(
        "TRN2",
        target_bir_lowering=False,
        debug=False,
        enable_asserts=False,
        num_devices=8,
    )
    pp_d = nc.dram_tensor("pp", [PP_TOTAL], BF, kind="ExternalInput").ap()
    wm_d = nc.dram_tensor("wm", [126, 256], BF, kind="ExternalInput").ap()
    out_d = nc.dram_tensor("out", [RPC, H, W * C], BF, kind="ExternalOutput").ap()

    with ExitStack() as ctx:
        tc = ctx.enter_context(tile.TileContext(nc))
        const_pool = ctx.enter_context(tc.tile_pool(name="const", bufs=1))
        slice_pool = ctx.enter_context(tc.tile_pool(name="slice", bufs=NS))
        t_pool = ctx.enter_context(tc.tile_pool(name="tt", bufs=8))
        ev_pool = ctx.enter_context(tc.tile_pool(name="ev", bufs=4))
        psum_pool = ctx.enter_context(tc.tile_pool(name="ps", bufs=4, space="PSUM"))

        # weights on the scalar ring so the sync ring is purely slice loads
        wm_sb = const_pool.tile([126, 256], BF)
        nc.scalar.dma_start(wm_sb[:], wm_d[:])

        # statically allocated slice tiles: all 9 loads issue immediately and
        # stream back-to-back on the sync ring at full HBM bandwidth
        def wfold(tk, nkd, ki, rr):
            """Two flat 512-wide DVE adds producing T[p, (pr, a, t, c)]:
            T = P[u=a, kw=t] + P[u=a-1, kw=t+2]; fully contiguous operands
            (wider/3D-AP variants measured 4-6x slower per element)."""
            T = t_pool.tile([126, 1024], BF, tag="T")
            for pr in range(2):
                base = pr * (1040 * nkd) + ki * 1040
                nc.vector.tensor_add(
                    T[:, pr * 512 : (pr + 1) * 512],
                    tk[:, base + 8 : base + 520],
                    tk[:, base + 520 : base + 1032],
                )
            return T

        def mm_half(ps, T, start, stop):
            # h-fold: K=(kh_lo, s)=126 packed; N=512 (PE max) per matmul
            for pr in range(2):
                nc.tensor.matmul(
                    ps[:],
                    wm_sb[:, pr * 128 : (pr + 1) * 128],
                    T[:, pr * 512 : (pr + 1) * 512],
                    start=start and pr == 0,
                    stop=stop and pr == 1,
                )

        # software pipeline over slices: when slice k lands, finish rows
        # (k-1 pair) with their A-half (adds + matmuls + evict + store), then
        # immediately start the B-half of the next rows so only the A-half of
        # the final two rows sits on the tail critical path.
        tiles = []
        pend = {}  # rr -> (ps, d_loc)
        for k in range(NS):
            w = _slice_width(k)
            t = slice_pool.tile([126, w], BF, tag="slice")
            src = pp_d[int(_OFFS[k]) : int(_OFFS[k]) + 126 * w].rearrange(
                "(p f) -> p f", f=w
            )
            nc.sync.dma_start(t[:], src)
            tiles.append(t)

            if k >= 1:
                nkd_A = 2 if k == NS - 1 else 4
                for rr in range(2):
                    ps, d_loc = pend[rr]
                    TA = wfold(tiles[k], nkd_A, rr, rr)
                    mm_half(ps, TA, start=False, stop=True)
                    ev = ev_pool.tile([128, 512], BF, tag="ev")
                    if rr == 0:
                        nc.scalar.copy(ev[:], ps[:])
                    else:
                        nc.vector.tensor_copy(ev[:], ps[:])
                    nc.scalar.dma_start(out_d[d_loc], ev[:])
            if k <= NS - 2:
                nkd_B = 2 if k == 0 else 4
                for rr in range(2):
                    ps = psum_pool.tile([128, 512], mybir.dt.float32, tag="ps")
                    kiB = rr if k == 0 else rr + 2
                    TB = wfold(tiles[k], nkd_B, kiB, rr)
                    mm_half(ps, TB, start=True, stop=False)
                    pend[rr] = (ps, 2 * k + rr)
    nc.compile()
    return nc


def _host_wm():
    rh = np.where(
        (np.arange(H) < 2) | (np.arange(H) >= H - 2), 1.0, 0.5
    ).astype(np.float32)
    wm = np.zeros((126, 256), np.float32)
    jj = np.arange(2)[:, None]
    s = np.arange(63)[None, :]
    for pr in range(2):
        h = (2 * s + 2 * pr + jj).ravel()
        wm[np.arange(126), pr * 128 + h] = 0.25 * rh[h]
    return wm.astype(ml_dtypes.bfloat16)


def _shard_inputs(patches):
    """Per-core flat bf16 patch blocks, 9 slices each in layout
    [p = kh_lo*63 + s, (kh_hi, kd, kw_pair, x=u+1, kw_lo, c)]."""
    P5 = np.asarray(patches, np.float32).reshape(B, od, oh, ow, 4, 4, 4, 4)
    P5 = P5.astype(ml_dtypes.bfloat16)
    pps = []
    for core in range(8):
        b, kc = core // 4, core % 4
        parts = []
        for k in range(NS):
            q = 8 * kc - 1 + k
            kdl = slice(2, 4) if k == 0 else slice(0, 2) if k == NS - 1 else slice(0, 4)
            nkd = 2 if k in (0, NS - 1) else 4
            arr = np.zeros((2, 63, 2, nkd, 2, X, 2, 4), ml_dtypes.bfloat16)
            if 0 <= q < od:
                src = P5[b, q, :, :, kdl]                      # s,u,kd',kh,kw,c
                s6 = src.reshape(63, 63, nkd, 2, 2, 2, 2, 4)   # s,u,kd,pr,jj,vg,vh,c
                arr[:, :, :, :, :, 1:64] = s6.transpose(4, 0, 3, 2, 5, 1, 6, 7)
            parts.append(arr.reshape(-1))
        pps.append(np.concatenate(parts))
    return pps


def _run(patches, trace=False):
    if "nc" not in _cache:
        _cache["nc"] = _build()
        _cache["wm"] = _host_wm()
    nc = _cache["nc"]
    wm = _cache["wm"]
    pps = _shard_inputs(patches)
    in_maps = [{"pp": pps[core], "wm": wm} for core in range(8)]
    res = bass_utils.run_bass_kernel_spmd(
        nc, in_maps, core_ids=list(range(8)), trace=trace
    )
    out = np.zeros((B, D, H, W, C), np.float32)
    for core in range(8):
        b, kc = core // 4, core % 4
        r = np.asarray(res.results[core]["out"]).astype(np.float32)
        out[b, RPC * kc : RPC * (kc + 1)] = r.reshape(RPC, H, W, C)
    out[:, [0, 1, D - 2, D - 1]] *= 2.0
    out[:, :, :, [0, 1, W - 2, W - 1], :] *= 2.0
    return out, res


def kernel(patches, inputs):
    out, _ = _run(patches)
    return out
